# revision 5
# baseline (speedup 1.0000x reference)
"""Trainium2 Bass kernel for nn_NodeNetwork (GNN message passing).

Computation (per batch b):
    bo = Ro^T X            [E, D]   gather  (contract n)
    bi = Ri^T X            [E, D]   gather  (contract n)
    mi = (Ri . e) bo       [N, D]   scatter (contract e)
    mo = (Ro . e) bi       [N, D]   scatter (contract e)
    h  = tanh([mi, mo, X] @ W1 + b1)
    y  = tanh(h @ W2 + b2)

The incidence matrices are one-hot over nodes (each edge has exactly one
in-node and one out-node), so the bmm contractions are really an indexed
gather and a weighted scatter-add:

    mi[n, :] = sum_{e: ri[e] = n} e_w[e] * X[ro[e], :]      (mo symmetric)

Sharding: 8 cores = 2 batches x 4 node-slices (NSL = N/4 = 1024 nodes per
core).  The host converts Ri/Ro to index form (argmax), groups each core's
incident edges by 128-node destination window (padded to a fixed 640 edges
per window with zero-weight edges), and ships only indices + weights.

On-device, per direction (mi/mo):
  - dma_gather fetches the 5120 per-edge source rows X[src] (fp16, 256B
    descriptors) straight into the [128e, blk, d] stationary layout.
  - For each 128-edge block, a one-hot scatter tile [128e, 128n] is built
    with a single DVE tensor_scalar: (iota == dest_rel) * weight.
  - matmul(ps[d, n_window] += G_blk^T @ onehot) accumulates the window's
    scatter in PSUM; 5 blocks per window, 8 windows per direction.
The MLP (two small dense layers + tanh) runs on the core's node slice.
No collectives: each core owns its output slice end-to-end.
"""

import numpy as np

import concourse.bass as bass
import concourse.mybir as mybir
import concourse.tile as tile
from concourse import bacc
from concourse.bass_utils import run_bass_kernel_spmd

B, N, E, D, OUT = 2, 4096, 16384, 64, 64
NCORES = 8
G = 4                  # cores per batch
NSL = N // G           # 1024 nodes per core
NW = NSL // 128        # 8 destination windows per core
WCAP = 640             # padded edges per window (5 blocks of 128)
BPW = WCAP // 128      # 5 blocks per window
EPC = NW * WCAP        # 5120 edges per direction per core
NBLK = 2 * EPC // 128  # 80 edge blocks (mi then mo)
NCHUNK = 10            # dma_gather split (HW limit: <=1024 idxs per gather)
CHUNK = 2 * EPC // NCHUNK  # 2560 indices per gather

F32 = mybir.dt.float32
F16 = mybir.dt.float16
I16 = mybir.dt.int16

_cache = {}


def _build_program(repeat=1):
    nc = bacc.Bacc(
        "TRN2",
        target_bir_lowering=False,
        debug=False,
        num_devices=NCORES,
    )

    xsrc = nc.declare_dram_parameter("xsrc", [N, 128], F16, isOutput=False)
    gidx = nc.declare_dram_parameter("gidx", [128, 2 * EPC // 16], I16, isOutput=False)
    dcol = nc.declare_dram_parameter("dcol", [128, NBLK], F32, isOutput=False)
    wcol = nc.declare_dram_parameter("wcol", [128, NBLK], F32, isOutput=False)
    iotap = nc.declare_dram_parameter("iotap", [128, 128], F16, isOutput=False)
    xtp = nc.declare_dram_parameter("xtp", [D, NSL], F16, isOutput=False)
    w1a = nc.declare_dram_parameter("w1a", [128, OUT], F16, isOutput=False)
    w1b = nc.declare_dram_parameter("w1b", [D, OUT], F16, isOutput=False)
    w2p = nc.declare_dram_parameter("w2p", [OUT, OUT], F16, isOutput=False)
    b1d = nc.declare_dram_parameter("b1d", [OUT, 1], F32, isOutput=False)
    b2d = nc.declare_dram_parameter("b2d", [OUT, 1], F32, isOutput=False)
    out = nc.declare_dram_parameter("out", [OUT, NSL], F32, isOutput=True)

    with tile.TileContext(nc) as tc:
        with (
            tc.tile_pool(name="const", bufs=1) as cpool,
            tc.tile_pool(name="oh", bufs=6) as ohpool,
            tc.tile_pool(name="st", bufs=2) as stpool,
            tc.tile_pool(name="pscat", bufs=4, space="PSUM") as pscat,
            tc.tile_pool(name="pmlp", bufs=2, space="PSUM") as pmlp,
        ):
            # ---- constants ----
            gidx_sb = cpool.tile([128, 2 * EPC // 16], I16)
            nc.sync.dma_start(gidx_sb[:], gidx[:])
            iota_sb = cpool.tile([128, 128], F16)
            nc.sync.dma_start(iota_sb[:], iotap[:])
            dcol_sb = cpool.tile([128, NBLK], F32)
            nc.sync.dma_start(dcol_sb[:], dcol[:])
            wcol_sb = cpool.tile([128, NBLK], F32)
            nc.sync.dma_start(wcol_sb[:], wcol[:])
            xt_sb = cpool.tile([D, NSL], F16)
            nc.sync.dma_start(xt_sb[:], xtp[:])
            w1a_sb = cpool.tile([128, OUT], F16)
            nc.sync.dma_start(w1a_sb[:], w1a[:])
            w1b_sb = cpool.tile([D, OUT], F16)
            nc.sync.dma_start(w1b_sb[:], w1b[:])
            w2_sb = cpool.tile([OUT, OUT], F16)
            nc.sync.dma_start(w2_sb[:], w2p[:])
            b1_sb = cpool.tile([OUT, 1], F32)
            nc.sync.dma_start(b1_sb[:], b1d[:])
            b2_sb = cpool.tile([OUT, 1], F32)
            nc.sync.dma_start(b2_sb[:], b2d[:])

            # gathered per-edge source rows (both directions)
            gat = cpool.tile([128, NBLK, 128], F16)

            # dummy gather before the loop so Bacc's library-reload pass
            # places the (expensive) GPSIMD mlp-library load in the preamble
            # instead of inside the repeat loop body.
            warm = cpool.tile([128, 1, 128], F16)
            nc.gpsimd.dma_gather(warm[:], xsrc[:], gidx_sb[:, :1], 16, 16, 128)

            def body(_i=None):
                for k in range(NCHUNK):
                    nb0 = k * (NBLK // NCHUNK)
                    nb1 = (k + 1) * (NBLK // NCHUNK)
                    nc.gpsimd.dma_gather(
                        gat[:, nb0:nb1, :],
                        xsrc[:],
                        gidx_sb[:, k * (CHUNK // 16) : (k + 1) * (CHUNK // 16)],
                        CHUNK,
                        CHUNK,
                        128,
                    )

                ps_mi = [
                    pscat.tile([64, 512], F32, tag="ps", name=f"psmi{h}")
                    for h in range(2)
                ]
                ps_mo = [
                    pscat.tile([64, 512], F32, tag="ps", name=f"psmo{h}")
                    for h in range(2)
                ]
                for b in range(NBLK):
                    dirb, bb = divmod(b, NBLK // 2)
                    w, r = divmod(bb, BPW)
                    half, wq = divmod(w, 4)
                    oh = ohpool.tile([128, 128], F16, tag="oh", name="oh")
                    nc.vector.tensor_scalar(
                        oh[:],
                        iota_sb[:],
                        dcol_sb[:, b : b + 1],
                        wcol_sb[:, b : b + 1],
                        mybir.AluOpType.is_equal,
                        mybir.AluOpType.mult,
                    )
                    ps = (ps_mi if dirb == 0 else ps_mo)[half]
                    nc.tensor.matmul(
                        ps[:, wq * 128 : (wq + 1) * 128],
                        gat[:, b, :D],
                        oh[:],
                        start=(r == 0),
                        stop=(r == BPW - 1),
                    )

                # ---- MLP on this core's node slice ----
                hA = stpool.tile([128, NSL], F16, tag="hA", name="hA")
                cp = mybir.ActivationFunctionType.Copy
                nc.scalar.activation(hA[:64, :512], ps_mi[0], cp)
                nc.scalar.activation(hA[:64, 512:], ps_mi[1], cp)
                nc.scalar.activation(hA[64:, :512], ps_mo[0], cp)
                nc.scalar.activation(hA[64:, 512:], ps_mo[1], cp)
                h2 = stpool.tile([OUT, NSL], F16, tag="h2", name="h2")
                for h in range(2):
                    sl = slice(h * 512, (h + 1) * 512)
                    pz = pmlp.tile([64, 512], F32, tag="pz", name="pz")
                    nc.tensor.matmul(pz, w1a_sb[:], hA[:, sl], start=True, stop=False)
                    nc.tensor.matmul(pz, w1b_sb[:], xt_sb[:, sl], start=False, stop=True)
                    nc.scalar.activation(
                        h2[:, sl], pz, mybir.ActivationFunctionType.Tanh, bias=b1_sb[:]
                    )
                for h in range(2):
                    sl = slice(h * 512, (h + 1) * 512)
                    py = pmlp.tile([64, 512], F32, tag="py", name="py")
                    nc.tensor.matmul(py, w2_sb[:], h2[:, sl], start=True, stop=True)
                    ysb = stpool.tile([64, 512], F32, tag="ysb", name="ysb")
                    nc.scalar.activation(
                        ysb[:], py, mybir.ActivationFunctionType.Tanh, bias=b2_sb[:]
                    )
                    nc.sync.dma_start(out[:, sl], ysb[:])

            if repeat == 1:
                body()
            else:
                with tc.For_i(0, repeat, 1) as _i:
                    body(_i)

    nc.compile()
    return nc


def make_in_maps(X, e, Ri, Ro, W1, b1, W2, b2):
    """Convert one-hot incidence to index form, shard by node slice."""
    X = np.asarray(X, dtype=np.float32)
    e = np.asarray(e, dtype=np.float32)
    W1 = np.asarray(W1, dtype=np.float32)
    b1 = np.asarray(b1, dtype=np.float32)
    W2 = np.asarray(W2, dtype=np.float32)
    b2 = np.asarray(b2, dtype=np.float32)
    ri = np.asarray(Ri, dtype=np.float32).argmax(axis=1)  # [B, E] in-node of edge
    ro = np.asarray(Ro, dtype=np.float32).argmax(axis=1)  # [B, E] out-node of edge

    w1a = W1[:128].astype(np.float16)
    w1b = W1[128 : 128 + D].astype(np.float16)
    w2c = W2.astype(np.float16)
    b1c = np.ascontiguousarray(b1.reshape(OUT, 1))
    b2c = np.ascontiguousarray(b2.reshape(OUT, 1))
    iota = np.broadcast_to(
        np.arange(128, dtype=np.float16)[None, :], (128, 128)
    ).copy()

    xsrc_b = {}
    for b_ in range(B):
        xs = np.zeros((N, 128), np.float16)
        xs[:, :D] = X[b_].astype(np.float16)
        xsrc_b[b_] = xs

    wrap = (np.arange(2 * EPC // 16)[None, :] * 16 + (np.arange(128) % 16)[:, None])

    in_maps = []
    for c in range(NCORES):
        b_, s = divmod(c, G)
        SRC = np.zeros(2 * EPC, np.int64)
        DC = np.zeros(2 * EPC, np.float32)
        WT = np.zeros(2 * EPC, np.float32)
        for d_i, (dest_all, src_all) in enumerate(
            ((ri[b_], ro[b_]), (ro[b_], ri[b_]))
        ):
            for w in range(NW):
                lo = s * NSL + w * 128
                idx = np.nonzero((dest_all >= lo) & (dest_all < lo + 128))[0]
                cnt = len(idx)
                assert cnt <= WCAP, f"window overflow: core {c} dir {d_i} w {w}: {cnt}"
                o = d_i * EPC + w * WCAP
                SRC[o : o + cnt] = src_all[idx]
                DC[o : o + cnt] = dest_all[idx] - lo
                WT[o : o + cnt] = e[b_, idx]
        in_maps.append(
            {
                "xsrc": xsrc_b[b_],
                "gidx": SRC[wrap].astype(np.int16),
                "dcol": np.ascontiguousarray(DC.reshape(NBLK, 128).T),
                "wcol": np.ascontiguousarray(WT.reshape(NBLK, 128).T),
                "iotap": iota,
                "xtp": np.ascontiguousarray(X[b_, s * NSL : (s + 1) * NSL, :].T).astype(
                    np.float16
                ),
                "w1a": w1a,
                "w1b": w1b,
                "w2p": w2c,
                "b1d": b1c,
                "b2d": b2c,
            }
        )
    return in_maps


def assemble_output(results):
    y = np.empty((B, N, OUT), dtype=np.float32)
    for c in range(NCORES):
        b_, s = divmod(c, G)
        y[b_, s * NSL : (s + 1) * NSL, :] = results[c]["out"].T
    return y


def get_program(repeat=1):
    key = ("nc", repeat)
    if key not in _cache:
        _cache[key] = _build_program(repeat)
    return _cache[key]


def kernel(X, e, Ri, Ro, W1, b1, W2, b2):
    nc = get_program()
    in_maps = make_in_maps(X, e, Ri, Ro, W1, b1, W2, b2)
    res = run_bass_kernel_spmd(nc, in_maps, list(range(NCORES)))
    return assemble_output(res.results)


# revision 8
# speedup vs baseline: 1.9561x; 1.9561x over previous
"""Trainium2 Bass kernel for nn_NodeNetwork (GNN message passing).

Computation (per batch b):
    bo = Ro^T X            [E, D]   gather  (contract n)
    bi = Ri^T X            [E, D]   gather  (contract n)
    mi = (Ri . e) bo       [N, D]   scatter (contract e)
    mo = (Ro . e) bi       [N, D]   scatter (contract e)
    h  = tanh([mi, mo, X] @ W1 + b1)
    y  = tanh(h @ W2 + b2)

The incidence matrices are one-hot over nodes (each edge has exactly one
in-node and one out-node), so the bmm contractions are really an indexed
gather and a weighted scatter-add:

    mi[n, :] = sum_{e: ri[e] = n} e_w[e] * X[ro[e], :]      (mo symmetric)

Sharding: 8 cores = 2 batches x 4 node-slices (NSL = N/4 = 1024 nodes per
core).  The host converts Ri/Ro to index form (argmax), groups each core's
incident edges by 128-node destination window (padded to a fixed 640 edges
per window with zero-weight edges), and ships only indices + weights.

On-device, per direction (mi/mo):
  - dma_gather fetches the 5120 per-edge source rows X[src] (fp16, 256B
    descriptors) straight into the [128e, blk, d] stationary layout.
  - For each 128-edge block, a one-hot scatter tile [128e, 128n] is built
    with a single DVE tensor_scalar: (iota == dest_rel) * weight.
  - matmul(ps[d, n_window] += G_blk^T @ onehot) accumulates the window's
    scatter in PSUM; 5 blocks per window, 8 windows per direction.
The MLP (two small dense layers + tanh) runs on the core's node slice.
No collectives: each core owns its output slice end-to-end.
"""

import numpy as np

import concourse.bass as bass
import concourse.mybir as mybir
import concourse.tile as tile
from concourse import bacc
from concourse.bass_utils import run_bass_kernel_spmd

B, N, E, D, OUT = 2, 4096, 16384, 64, 64
NCORES = 8
G = 4                  # cores per batch
NSL = N // G           # 1024 nodes per core
NW = NSL // 128        # 8 destination windows per core
WCAP = 640             # padded edges per window (5 blocks of 128)
BPW = WCAP // 128      # 5 blocks per window
EPC = NW * WCAP        # 5120 edges per direction per core
NBLK = 2 * EPC // 128  # 80 edge blocks (mi then mo)
NCHUNK = 10            # dma_gather split (HW limit: <=1024 idxs per gather)
CHUNK = 2 * EPC // NCHUNK  # 2560 indices per gather

F32 = mybir.dt.float32
F16 = mybir.dt.float16
I16 = mybir.dt.int16

_cache = {}


def _build_program(repeat=1):
    nc = bacc.Bacc(
        "TRN2",
        target_bir_lowering=False,
        debug=False,
        num_devices=NCORES,
        num_swdge_queues=4,
    )

    xsrc = nc.declare_dram_parameter("xsrc", [N, 128], F16, isOutput=False)
    gidx = nc.declare_dram_parameter("gidx", [128, 2 * EPC // 16], I16, isOutput=False)
    dcol = nc.declare_dram_parameter("dcol", [128, NBLK], F32, isOutput=False)
    wcol = nc.declare_dram_parameter("wcol", [128, NBLK], F32, isOutput=False)
    iotap = nc.declare_dram_parameter("iotap", [128, 128], F16, isOutput=False)
    xtp = nc.declare_dram_parameter("xtp", [D, NSL], F16, isOutput=False)
    w1a = nc.declare_dram_parameter("w1a", [128, OUT], F16, isOutput=False)
    w1b = nc.declare_dram_parameter("w1b", [D, OUT], F16, isOutput=False)
    w2p = nc.declare_dram_parameter("w2p", [OUT, OUT], F16, isOutput=False)
    b1d = nc.declare_dram_parameter("b1d", [OUT, 1], F32, isOutput=False)
    b2d = nc.declare_dram_parameter("b2d", [OUT, 1], F32, isOutput=False)
    out = nc.declare_dram_parameter("out", [OUT, NSL], F32, isOutput=True)

    with tile.TileContext(nc) as tc:
        with (
            tc.tile_pool(name="const", bufs=1) as cpool,
            tc.tile_pool(name="oh", bufs=6) as ohpool,
            tc.tile_pool(name="st", bufs=2) as stpool,
            tc.tile_pool(name="pscat", bufs=4, space="PSUM") as pscat,
            tc.tile_pool(name="pmlp", bufs=2, space="PSUM") as pmlp,
        ):
            # ---- constants ----
            gidx_sb = cpool.tile([128, 2 * EPC // 16], I16)
            nc.sync.dma_start(gidx_sb[:], gidx[:])
            iota_sb = cpool.tile([128, 128], F16)
            nc.sync.dma_start(iota_sb[:], iotap[:])
            dcol_sb = cpool.tile([128, NBLK], F32)
            nc.sync.dma_start(dcol_sb[:], dcol[:])
            wcol_sb = cpool.tile([128, NBLK], F32)
            nc.sync.dma_start(wcol_sb[:], wcol[:])
            xt_sb = cpool.tile([D, NSL], F16)
            nc.sync.dma_start(xt_sb[:], xtp[:])
            w1a_sb = cpool.tile([128, OUT], F16)
            nc.sync.dma_start(w1a_sb[:], w1a[:])
            w1b_sb = cpool.tile([D, OUT], F16)
            nc.sync.dma_start(w1b_sb[:], w1b[:])
            w2_sb = cpool.tile([OUT, OUT], F16)
            nc.sync.dma_start(w2_sb[:], w2p[:])
            b1_sb = cpool.tile([OUT, 1], F32)
            nc.sync.dma_start(b1_sb[:], b1d[:])
            b2_sb = cpool.tile([OUT, 1], F32)
            nc.sync.dma_start(b2_sb[:], b2d[:])

            # gathered per-edge source rows (both directions)
            gat = cpool.tile([128, NBLK, 128], F16)

            # dummy gather before the loop so Bacc's library-reload pass
            # places the (expensive) GPSIMD mlp-library load in the preamble
            # instead of inside the repeat loop body.
            warm = cpool.tile([128, 1, 128], F16)
            nc.gpsimd.dma_gather(warm[:], xsrc[:], gidx_sb[:, :1], 16, 16, 128)

            def body(_i=None):
                for k in range(NCHUNK):
                    nb0 = k * (NBLK // NCHUNK)
                    nb1 = (k + 1) * (NBLK // NCHUNK)
                    nc.gpsimd.dma_gather(
                        gat[:, nb0:nb1, :],
                        xsrc[:],
                        gidx_sb[:, k * (CHUNK // 16) : (k + 1) * (CHUNK // 16)],
                        CHUNK,
                        CHUNK,
                        128,
                        queue_num=(1 + k) % 4,
                    )

                ps_mi = [
                    pscat.tile([64, 512], F32, tag="ps", name=f"psmi{h}")
                    for h in range(2)
                ]
                ps_mo = [
                    pscat.tile([64, 512], F32, tag="ps", name=f"psmo{h}")
                    for h in range(2)
                ]
                for b in range(NBLK):
                    dirb, bb = divmod(b, NBLK // 2)
                    w, r = divmod(bb, BPW)
                    half, wq = divmod(w, 4)
                    oh = ohpool.tile([128, 128], F16, tag="oh", name="oh")
                    nc.vector.tensor_scalar(
                        oh[:],
                        iota_sb[:],
                        dcol_sb[:, b : b + 1],
                        wcol_sb[:, b : b + 1],
                        mybir.AluOpType.is_equal,
                        mybir.AluOpType.mult,
                    )
                    ps = (ps_mi if dirb == 0 else ps_mo)[half]
                    nc.tensor.matmul(
                        ps[:, wq * 128 : (wq + 1) * 128],
                        gat[:, b, :D],
                        oh[:],
                        start=(r == 0),
                        stop=(r == BPW - 1),
                    )

                # ---- MLP on this core's node slice ----
                hA = stpool.tile([128, NSL], F16, tag="hA", name="hA")
                cp = mybir.ActivationFunctionType.Copy
                nc.scalar.activation(hA[:64, :512], ps_mi[0], cp)
                nc.scalar.activation(hA[:64, 512:], ps_mi[1], cp)
                nc.scalar.activation(hA[64:, :512], ps_mo[0], cp)
                nc.scalar.activation(hA[64:, 512:], ps_mo[1], cp)
                h2 = stpool.tile([OUT, NSL], F16, tag="h2", name="h2")
                for h in range(2):
                    sl = slice(h * 512, (h + 1) * 512)
                    pz = pmlp.tile([64, 512], F32, tag="pz", name="pz")
                    nc.tensor.matmul(pz, w1a_sb[:], hA[:, sl], start=True, stop=False)
                    nc.tensor.matmul(pz, w1b_sb[:], xt_sb[:, sl], start=False, stop=True)
                    nc.scalar.activation(
                        h2[:, sl], pz, mybir.ActivationFunctionType.Tanh, bias=b1_sb[:]
                    )
                for h in range(2):
                    sl = slice(h * 512, (h + 1) * 512)
                    py = pmlp.tile([64, 512], F32, tag="py", name="py")
                    nc.tensor.matmul(py, w2_sb[:], h2[:, sl], start=True, stop=True)
                    ysb = stpool.tile([64, 512], F32, tag="ysb", name="ysb")
                    nc.scalar.activation(
                        ysb[:], py, mybir.ActivationFunctionType.Tanh, bias=b2_sb[:]
                    )
                    nc.sync.dma_start(out[:, sl], ysb[:])

            if repeat == 1:
                body()
            else:
                with tc.For_i(0, repeat, 1) as _i:
                    body(_i)

    nc.compile()
    return nc


def make_in_maps(X, e, Ri, Ro, W1, b1, W2, b2):
    """Convert one-hot incidence to index form, shard by node slice."""
    X = np.asarray(X, dtype=np.float32)
    e = np.asarray(e, dtype=np.float32)
    W1 = np.asarray(W1, dtype=np.float32)
    b1 = np.asarray(b1, dtype=np.float32)
    W2 = np.asarray(W2, dtype=np.float32)
    b2 = np.asarray(b2, dtype=np.float32)
    ri = np.asarray(Ri, dtype=np.float32).argmax(axis=1)  # [B, E] in-node of edge
    ro = np.asarray(Ro, dtype=np.float32).argmax(axis=1)  # [B, E] out-node of edge

    w1a = W1[:128].astype(np.float16)
    w1b = W1[128 : 128 + D].astype(np.float16)
    w2c = W2.astype(np.float16)
    b1c = np.ascontiguousarray(b1.reshape(OUT, 1))
    b2c = np.ascontiguousarray(b2.reshape(OUT, 1))
    iota = np.broadcast_to(
        np.arange(128, dtype=np.float16)[None, :], (128, 128)
    ).copy()

    xsrc_b = {}
    for b_ in range(B):
        xs = np.zeros((N, 128), np.float16)
        xs[:, :D] = X[b_].astype(np.float16)
        xsrc_b[b_] = xs

    wrap = (np.arange(2 * EPC // 16)[None, :] * 16 + (np.arange(128) % 16)[:, None])

    in_maps = []
    for c in range(NCORES):
        b_, s = divmod(c, G)
        SRC = np.zeros(2 * EPC, np.int64)
        DC = np.zeros(2 * EPC, np.float32)
        WT = np.zeros(2 * EPC, np.float32)
        for d_i, (dest_all, src_all) in enumerate(
            ((ri[b_], ro[b_]), (ro[b_], ri[b_]))
        ):
            for w in range(NW):
                lo = s * NSL + w * 128
                idx = np.nonzero((dest_all >= lo) & (dest_all < lo + 128))[0]
                cnt = len(idx)
                assert cnt <= WCAP, f"window overflow: core {c} dir {d_i} w {w}: {cnt}"
                o = d_i * EPC + w * WCAP
                SRC[o : o + cnt] = src_all[idx]
                DC[o : o + cnt] = dest_all[idx] - lo
                WT[o : o + cnt] = e[b_, idx]
        in_maps.append(
            {
                "xsrc": xsrc_b[b_],
                "gidx": SRC[wrap].astype(np.int16),
                "dcol": np.ascontiguousarray(DC.reshape(NBLK, 128).T),
                "wcol": np.ascontiguousarray(WT.reshape(NBLK, 128).T),
                "iotap": iota,
                "xtp": np.ascontiguousarray(X[b_, s * NSL : (s + 1) * NSL, :].T).astype(
                    np.float16
                ),
                "w1a": w1a,
                "w1b": w1b,
                "w2p": w2c,
                "b1d": b1c,
                "b2d": b2c,
            }
        )
    return in_maps


def assemble_output(results):
    y = np.empty((B, N, OUT), dtype=np.float32)
    for c in range(NCORES):
        b_, s = divmod(c, G)
        y[b_, s * NSL : (s + 1) * NSL, :] = results[c]["out"].T
    return y


def get_program(repeat=1):
    key = ("nc", repeat)
    if key not in _cache:
        _cache[key] = _build_program(repeat)
    return _cache[key]


def kernel(X, e, Ri, Ro, W1, b1, W2, b2):
    nc = get_program()
    in_maps = make_in_maps(X, e, Ri, Ro, W1, b1, W2, b2)
    res = run_bass_kernel_spmd(nc, in_maps, list(range(NCORES)))
    return assemble_output(res.results)
